# revision 4
# baseline (speedup 1.0000x reference)
"""MoE layer (E=8 experts, top-2 routing) on 8 Trainium2 NeuronCores.

Strategy (expert-parallel, per the sharding hint):
  - The gate (T x D @ D x E, softmax, top-2, renorm) is computed on the host
    in fp32; it is ~0.01% of the FLOPs.
  - Tokens are dispatched by expert id ("all-to-all" done host-side): core e
    receives the tokens routed to expert e (padded to a common capacity C),
    together with expert e's weights in bf16.
  - Each core runs a Bass/Tile kernel computing
        y = combine_weight * (gelu(x @ w1 + b1) @ w2 + b2)
    with bf16 matmuls (fp32 PSUM accumulation) on the PE array:
      * phase H: H^T tiles (feature-major) = w1-chunk^T.T @ x^T-chunk,
        so no on-device transposes are needed (w1 natural layout is lhsT).
      * phase Y: token-major Y = H^T-chunk.T @ w2-chunk, which makes the
        per-token combine weight a per-partition scalar.
  - Host "unshard" is two gathers + an add (each token has exactly 2 slots).
"""

import numpy as np
import ml_dtypes

import concourse.bass as bass
import concourse.mybir as mybir
from concourse import bacc
from concourse.tile import TileContext
from concourse.bass_utils import run_bass_kernel_spmd

P = 128
D = 1024
F = 4096
E = 8
TOPK = 2
NBLK = 512

_BF16 = ml_dtypes.bfloat16

_nc_cache: dict = {}
LAST = None  # BassKernelResults of the most recent run (for test harness)


def _build_moe_core(C: int) -> bass.Bass:
    """One-core SPMD program: FFN for C tokens with resident bf16 weights."""
    dt = mybir.dt
    nc = bacc.Bacc("TRN2", target_bir_lowering=False, debug=False)
    xt = nc.dram_tensor("xt", [D, C], dt.bfloat16, kind="ExternalInput")
    w1 = nc.dram_tensor("w1", [D, F], dt.bfloat16, kind="ExternalInput")
    w2 = nc.dram_tensor("w2", [F, D], dt.bfloat16, kind="ExternalInput")
    b1 = nc.dram_tensor("b1", [F], dt.float32, kind="ExternalInput")
    b2r = nc.dram_tensor("b2r", [P, D], dt.float32, kind="ExternalInput")
    sc = nc.dram_tensor("sc", [C], dt.float32, kind="ExternalInput")
    y = nc.dram_tensor("y", [C, D], dt.float32, kind="ExternalOutput")

    KO = D // P    # 8 contraction chunks for x @ w1
    FO = F // P    # 32 contraction chunks for h @ w2
    DN = D // NBLK  # 2 output-column blocks of w2
    GELU = mybir.ActivationFunctionType.Gelu

    blocks = []
    off = 0
    while off < C:
        size = min(NBLK, C - off)
        blocks.append((off, size))
        off += size

    xt_r = xt.rearrange("(ko p) c -> p ko c", p=P)

    with TileContext(nc) as tc:
        with (
            tc.tile_pool(name="w", bufs=1) as wpool,
            tc.tile_pool(name="xin", bufs=2) as xpool,
            tc.tile_pool(name="h", bufs=1) as hpool,
            tc.tile_pool(name="yout", bufs=2) as ypool,
            tc.tile_pool(name="ph", bufs=2, space="PSUM") as phpool,
            tc.tile_pool(name="py", bufs=4, space="PSUM") as pypool,
        ):
            # Per-chunk weight tiles so the first matmuls only wait on the
            # first chunk's DMA, not the whole 8MB load.
            w1sb = []
            for ko in range(KO):
                t_ = wpool.tile([P, F], dt.bfloat16, tag=f"w1_{ko}")
                nc.sync.dma_start(t_[:], w1[ko * P:(ko + 1) * P, :])
                w1sb.append(t_)
            w2sb = []
            for fo in range(FO):
                t_ = wpool.tile([P, D], dt.bfloat16, tag=f"w2_{fo}")
                nc.sync.dma_start(t_[:], w2[fo * P:(fo + 1) * P, :])
                w2sb.append(t_)
            b1sb = wpool.tile([P, FO], dt.float32, tag="b1")
            nc.sync.dma_start(b1sb[:], b1.rearrange("(fo p) -> p fo", p=P))
            b2sb = wpool.tile([P, D], dt.float32, tag="b2")
            nc.sync.dma_start(b2sb[:], b2r[:])
            scsb = wpool.tile([P, C // P], dt.float32, tag="sc")
            nc.sync.dma_start(scsb[:], sc.rearrange("(tb p) -> p tb", p=P))

            for (n_off, n_size) in blocks:
                xts = []
                for ko in range(KO):
                    xt_t = xpool.tile([P, NBLK], dt.bfloat16, tag=f"x_{ko}")
                    nc.sync.dma_start(
                        xt_t[:, :n_size], xt_r[:, ko, n_off:n_off + n_size]
                    )
                    xts.append(xt_t)

                # H^T[f, t] = sum_d w1[d, f] * x^T[d, t], then gelu(+b1).
                htile = hpool.tile([P, FO, NBLK], dt.bfloat16, tag="h")
                for fo in range(FO):
                    ph = phpool.tile([P, NBLK], dt.float32, tag="ph")
                    for ko in range(KO):
                        nc.tensor.matmul(
                            ph[:, :n_size],
                            w1sb[ko][:, fo * P:(fo + 1) * P],
                            xts[ko][:, :n_size],
                            start=(ko == 0),
                            stop=(ko == KO - 1),
                        )
                    nc.scalar.activation(
                        htile[:, fo, :n_size], ph[:, :n_size], GELU,
                        bias=b1sb[:, fo:fo + 1], scale=1.0,
                    )

                # Y[t, d] = sum_f H[t, f] * w2[f, d]; scale per token.
                for tb in range(n_size // P):
                    tbg = (n_off + tb * P) // P
                    ytile = ypool.tile([P, D], dt.float32, tag="y")
                    for dn in range(DN):
                        py = pypool.tile([P, NBLK], dt.float32, tag="py")
                        for fo in range(FO):
                            nc.tensor.matmul(
                                py[:],
                                htile[:, fo, tb * P:(tb + 1) * P],
                                w2sb[fo][:, dn * NBLK:(dn + 1) * NBLK],
                                start=(fo == 0),
                                stop=(fo == FO - 1),
                            )
                        nc.vector.tensor_add(
                            ytile[:, dn * NBLK:(dn + 1) * NBLK],
                            py[:],
                            b2sb[:, dn * NBLK:(dn + 1) * NBLK],
                        )
                    nc.vector.tensor_scalar_mul(
                        ytile[:], ytile[:], scsb[:, tbg:tbg + 1]
                    )
                    nc.sync.dma_start(
                        y[n_off + tb * P:n_off + (tb + 1) * P, :], ytile[:]
                    )
    nc.compile()
    return nc


def _route(flat, gate_w, gate_b):
    """fp32 gate matching the reference: softmax, top-2, renormalize."""
    logits = flat @ gate_w + gate_b
    m = logits.max(axis=1, keepdims=True)
    p = np.exp(logits - m, dtype=np.float32)
    probs = p / p.sum(axis=1, keepdims=True)
    ti = np.argsort(-probs, axis=1, kind="stable")[:, :TOPK]
    tp = np.take_along_axis(probs, ti, axis=1)
    sw = tp / (tp.sum(axis=1, keepdims=True) + np.float32(1e-9))
    return ti.astype(np.int64), sw.astype(np.float32)


def _dispatch(ti):
    """Slot assignment: (token, k) pair -> (expert, position-in-expert)."""
    Tn = ti.shape[0]
    flat_e = ti.ravel()
    order = np.argsort(flat_e, kind="stable")
    cnt = np.bincount(flat_e, minlength=E)
    starts = np.concatenate([[0], np.cumsum(cnt)[:-1]])
    ranks = np.arange(Tn * TOPK) - starts[flat_e[order]]
    pos = np.empty(Tn * TOPK, np.int64)
    pos[order] = ranks
    return flat_e, pos, cnt, starts, order


def kernel(**inputs) -> np.ndarray:
    global LAST
    x = np.asarray(inputs["x"], np.float32)
    gate_w = np.asarray(inputs["gate_w"], np.float32)
    gate_b = np.asarray(inputs["gate_b"], np.float32)
    w1 = np.asarray(inputs["w1"], np.float32)
    b1 = np.asarray(inputs["b1"], np.float32)
    w2 = np.asarray(inputs["w2"], np.float32)
    b2 = np.asarray(inputs["b2"], np.float32)

    B, S, D_ = x.shape
    flat = x.reshape(-1, D_)

    ti, sw = _route(flat, gate_w, gate_b)
    flat_e, pos, cnt, starts, order = _dispatch(ti)

    C = ((int(cnt.max()) + P - 1) // P) * P
    C = max(C, P)

    xT_bf = np.ascontiguousarray(flat.T).astype(_BF16)  # [D, T]
    sw_flat = sw.ravel()

    in_maps = []
    for e in range(E):
        pairs = order[starts[e]:starts[e] + cnt[e]]
        toks = pairs // TOPK
        xt_e = np.zeros((D, C), _BF16)
        xt_e[:, :cnt[e]] = xT_bf[:, toks]
        sc_e = np.zeros((C,), np.float32)
        sc_e[:cnt[e]] = sw_flat[pairs]
        in_maps.append({
            "xt": xt_e,
            "w1": w1[e].astype(_BF16),
            "w2": w2[e].astype(_BF16),
            "b1": np.ascontiguousarray(b1[e]),
            "b2r": np.ascontiguousarray(
                np.broadcast_to(b2[e], (P, D))
            ).astype(np.float32),
            "sc": sc_e,
        })

    nc = _nc_cache.get(C)
    if nc is None:
        nc = _build_moe_core(C)
        _nc_cache[C] = nc

    LAST = run_bass_kernel_spmd(nc, in_maps, core_ids=list(range(E)))
    Yall = np.stack([np.asarray(LAST.results[i]["y"]) for i in range(E)])

    contrib = Yall[flat_e, pos]  # [T*K, D] gather
    out = contrib[0::TOPK] + contrib[1::TOPK]
    return out.reshape(B, S, D_).astype(np.float32)


# revision 6
# speedup vs baseline: 1.0866x; 1.0866x over previous
"""MoE layer (E=8 experts, top-2 routing) on 8 Trainium2 NeuronCores.

Strategy (expert-parallel, per the sharding hint):
  - The gate (T x D @ D x E, softmax, top-2, renorm) is computed on the host
    in fp32; it is ~0.01% of the FLOPs.
  - Tokens are dispatched by expert id ("all-to-all" done host-side): core e
    receives the tokens routed to expert e (padded to a common capacity C),
    together with expert e's weights in bf16.
  - Each core runs a Bass/Tile kernel computing
        y = combine_weight * (gelu(x @ w1 + b1) @ w2 + b2)
    with bf16 matmuls (fp32 PSUM accumulation) on the PE array:
      * phase H: H^T tiles (feature-major) = w1-chunk^T.T @ x^T-chunk,
        so no on-device transposes are needed (w1 natural layout is lhsT).
      * phase Y: token-major Y = H^T-chunk.T @ w2-chunk, which makes the
        per-token combine weight a per-partition scalar.
  - Host "unshard" is two gathers + an add (each token has exactly 2 slots).
"""

import numpy as np
import ml_dtypes

import concourse.bass as bass
import concourse.mybir as mybir
from concourse import bacc
from concourse.tile import TileContext
from concourse.bass_utils import run_bass_kernel_spmd

P = 128
D = 1024
F = 4096
E = 8
TOPK = 2
NBLK = 512

_BF16 = ml_dtypes.bfloat16

_nc_cache: dict = {}
LAST = None  # BassKernelResults of the most recent run (for test harness)


def _build_moe_core(C: int) -> bass.Bass:
    """One-core SPMD program: FFN for C tokens with resident bf16 weights."""
    dt = mybir.dt
    nc = bacc.Bacc("TRN2", target_bir_lowering=False, debug=False)
    KO = D // P    # 8 contraction chunks for x @ w1
    FO = F // P    # 32 contraction chunks for h @ w2
    DN = D // NBLK  # 2 output-column blocks of w2
    GELU = mybir.ActivationFunctionType.Gelu

    xt = nc.dram_tensor("xt", [D, C], dt.bfloat16, kind="ExternalInput")
    # w1 host-pretiled per-fo: w1t[fo, p, ko, j] = w1[ko*P+p, fo*P+j], so each
    # 256KB fo-tile is one contiguous-per-partition DMA and the PE can start
    # after the first tile instead of the full 8MB.
    w1t = nc.dram_tensor("w1t", [FO, P, KO, P], dt.bfloat16,
                         kind="ExternalInput")
    w2 = nc.dram_tensor("w2", [F, D], dt.bfloat16, kind="ExternalInput")
    b1 = nc.dram_tensor("b1", [F], dt.float32, kind="ExternalInput")
    b2r = nc.dram_tensor("b2r", [P, D], dt.float32, kind="ExternalInput")
    sc = nc.dram_tensor("sc", [C], dt.float32, kind="ExternalInput")
    y = nc.dram_tensor("y", [C, D], dt.float32, kind="ExternalOutput")

    blocks = []
    off = 0
    while off < C:
        size = min(NBLK, C - off)
        blocks.append((off, size))
        off += size

    xt_r = xt.rearrange("(ko p) c -> p ko c", p=P)

    with TileContext(nc) as tc:
        with (
            tc.tile_pool(name="w", bufs=1) as wpool,
            tc.tile_pool(name="xin", bufs=2) as xpool,
            tc.tile_pool(name="h", bufs=1) as hpool,
            tc.tile_pool(name="yout", bufs=2) as ypool,
            tc.tile_pool(name="ph", bufs=2, space="PSUM") as phpool,
            tc.tile_pool(name="py", bufs=4, space="PSUM") as pypool,
        ):
            # DMA issue order is the startup critical path: tiny consts,
            # then x for block 0, then w1 fo-tiles (in use order), then w2.
            b1sb = wpool.tile([P, FO], dt.float32, tag="b1")
            nc.sync.dma_start(b1sb[:], b1.rearrange("(fo p) -> p fo", p=P))
            b2sb = wpool.tile([P, D], dt.float32, tag="b2")
            nc.sync.dma_start(b2sb[:], b2r[:])
            scsb = wpool.tile([P, C // P], dt.float32, tag="sc")
            nc.sync.dma_start(scsb[:], sc.rearrange("(tb p) -> p tb", p=P))

            def load_x_block(n_off, n_size):
                xts = []
                for ko in range(KO):
                    xt_t = xpool.tile([P, NBLK], dt.bfloat16, tag=f"x_{ko}")
                    nc.sync.dma_start(
                        xt_t[:, :n_size], xt_r[:, ko, n_off:n_off + n_size]
                    )
                    xts.append(xt_t)
                return xts

            xts0 = load_x_block(*blocks[0])

            w1sb = []
            for fo in range(FO):
                t_ = wpool.tile([P, KO, P], dt.bfloat16, tag=f"w1_{fo}")
                nc.sync.dma_start(t_[:], w1t[fo])
                w1sb.append(t_)
            w2sb = []
            for fo in range(FO):
                t_ = wpool.tile([P, D], dt.bfloat16, tag=f"w2_{fo}")
                nc.sync.dma_start(t_[:], w2[fo * P:(fo + 1) * P, :])
                w2sb.append(t_)

            for bi, (n_off, n_size) in enumerate(blocks):
                xts = xts0 if bi == 0 else load_x_block(n_off, n_size)

                # H^T[f, t] = sum_d w1[d, f] * x^T[d, t], then gelu(+b1).
                htile = hpool.tile([P, FO, NBLK], dt.bfloat16, tag="h")
                for fo in range(FO):
                    ph = phpool.tile([P, NBLK], dt.float32, tag="ph")
                    for ko in range(KO):
                        nc.tensor.matmul(
                            ph[:, :n_size],
                            w1sb[fo][:, ko, :],
                            xts[ko][:, :n_size],
                            start=(ko == 0),
                            stop=(ko == KO - 1),
                        )
                    nc.scalar.activation(
                        htile[:, fo, :n_size], ph[:, :n_size], GELU,
                        bias=b1sb[:, fo:fo + 1], scale=1.0,
                    )

                # Y[t, d] = sum_f H[t, f] * w2[f, d]; scale per token.
                for tb in range(n_size // P):
                    tbg = (n_off + tb * P) // P
                    ytile = ypool.tile([P, D], dt.float32, tag="y")
                    for dn in range(DN):
                        py = pypool.tile([P, NBLK], dt.float32, tag="py")
                        for fo in range(FO):
                            nc.tensor.matmul(
                                py[:],
                                htile[:, fo, tb * P:(tb + 1) * P],
                                w2sb[fo][:, dn * NBLK:(dn + 1) * NBLK],
                                start=(fo == 0),
                                stop=(fo == FO - 1),
                            )
                        nc.vector.tensor_add(
                            ytile[:, dn * NBLK:(dn + 1) * NBLK],
                            py[:],
                            b2sb[:, dn * NBLK:(dn + 1) * NBLK],
                        )
                    nc.vector.tensor_scalar_mul(
                        ytile[:], ytile[:], scsb[:, tbg:tbg + 1]
                    )
                    nc.sync.dma_start(
                        y[n_off + tb * P:n_off + (tb + 1) * P, :], ytile[:]
                    )
    nc.compile()
    return nc


def _route(flat, gate_w, gate_b):
    """fp32 gate matching the reference: softmax, top-2, renormalize."""
    logits = flat @ gate_w + gate_b
    m = logits.max(axis=1, keepdims=True)
    p = np.exp(logits - m, dtype=np.float32)
    probs = p / p.sum(axis=1, keepdims=True)
    ti = np.argsort(-probs, axis=1, kind="stable")[:, :TOPK]
    tp = np.take_along_axis(probs, ti, axis=1)
    sw = tp / (tp.sum(axis=1, keepdims=True) + np.float32(1e-9))
    return ti.astype(np.int64), sw.astype(np.float32)


def _dispatch(ti):
    """Slot assignment: (token, k) pair -> (expert, position-in-expert)."""
    Tn = ti.shape[0]
    flat_e = ti.ravel()
    order = np.argsort(flat_e, kind="stable")
    cnt = np.bincount(flat_e, minlength=E)
    starts = np.concatenate([[0], np.cumsum(cnt)[:-1]])
    ranks = np.arange(Tn * TOPK) - starts[flat_e[order]]
    pos = np.empty(Tn * TOPK, np.int64)
    pos[order] = ranks
    return flat_e, pos, cnt, starts, order


def kernel(**inputs) -> np.ndarray:
    global LAST
    x = np.asarray(inputs["x"], np.float32)
    gate_w = np.asarray(inputs["gate_w"], np.float32)
    gate_b = np.asarray(inputs["gate_b"], np.float32)
    w1 = np.asarray(inputs["w1"], np.float32)
    b1 = np.asarray(inputs["b1"], np.float32)
    w2 = np.asarray(inputs["w2"], np.float32)
    b2 = np.asarray(inputs["b2"], np.float32)

    B, S, D_ = x.shape
    flat = x.reshape(-1, D_)

    ti, sw = _route(flat, gate_w, gate_b)
    flat_e, pos, cnt, starts, order = _dispatch(ti)

    C = ((int(cnt.max()) + P - 1) // P) * P
    C = max(C, P)

    xT_bf = np.ascontiguousarray(flat.T).astype(_BF16)  # [D, T]
    sw_flat = sw.ravel()

    in_maps = []
    for e in range(E):
        pairs = order[starts[e]:starts[e] + cnt[e]]
        toks = pairs // TOPK
        xt_e = np.zeros((D, C), _BF16)
        xt_e[:, :cnt[e]] = xT_bf[:, toks]
        sc_e = np.zeros((C,), np.float32)
        sc_e[:cnt[e]] = sw_flat[pairs]
        KO, FO = D // P, F // P
        w1_tiled = np.ascontiguousarray(
            w1[e].astype(_BF16).reshape(KO, P, FO, P).transpose(2, 1, 0, 3)
        )
        in_maps.append({
            "xt": xt_e,
            "w1t": w1_tiled,
            "w2": w2[e].astype(_BF16),
            "b1": np.ascontiguousarray(b1[e]),
            "b2r": np.ascontiguousarray(
                np.broadcast_to(b2[e], (P, D))
            ).astype(np.float32),
            "sc": sc_e,
        })

    nc = _nc_cache.get(C)
    if nc is None:
        nc = _build_moe_core(C)
        _nc_cache[C] = nc

    LAST = run_bass_kernel_spmd(nc, in_maps, core_ids=list(range(E)))
    Yall = np.stack([np.asarray(LAST.results[i]["y"]) for i in range(E)])

    contrib = Yall[flat_e, pos]  # [T*K, D] gather
    out = contrib[0::TOPK] + contrib[1::TOPK]
    return out.reshape(B, S, D_).astype(np.float32)


# revision 9
# speedup vs baseline: 1.0967x; 1.0093x over previous
"""MoE layer (E=8 experts, top-2 routing) on 8 Trainium2 NeuronCores.

Strategy (expert-parallel, per the sharding hint):
  - The gate (T x D @ D x E, softmax, top-2, renorm) is computed on the host
    in fp32; it is ~0.01% of the FLOPs.
  - Tokens are dispatched by expert id ("all-to-all" done host-side): core e
    receives the tokens routed to expert e (padded to a common capacity C),
    together with expert e's weights in bf16.
  - Each core runs a Bass/Tile kernel computing
        y = combine_weight * (gelu(x @ w1 + b1) @ w2 + b2)
    with bf16 matmuls (fp32 PSUM accumulation) on the PE array:
      * phase H: H^T tiles (feature-major) = w1-chunk^T.T @ x^T-chunk,
        so no on-device transposes are needed (w1 natural layout is lhsT).
      * phase Y: token-major Y = H^T-chunk.T @ w2-chunk, which makes the
        per-token combine weight a per-partition scalar.
  - Host "unshard" is two gathers + an add (each token has exactly 2 slots).
"""

import numpy as np
import ml_dtypes

import concourse.bass as bass
import concourse.mybir as mybir
from concourse import bacc
from concourse.tile import TileContext
from concourse.bass_utils import run_bass_kernel_spmd

P = 128
D = 1024
F = 4096
E = 8
TOPK = 2
NBLK = 512

_BF16 = ml_dtypes.bfloat16

_nc_cache: dict = {}
LAST = None  # BassKernelResults of the most recent run (for test harness)


def _build_moe_core(C: int) -> bass.Bass:
    """One-core SPMD program: FFN for C tokens with resident bf16 weights."""
    dt = mybir.dt
    nc = bacc.Bacc("TRN2", target_bir_lowering=False, debug=False)
    KO = D // P    # 8 contraction chunks for x @ w1
    FO = F // P    # 32 contraction chunks for h @ w2
    DN = D // NBLK  # 2 output-column blocks of w2
    GELU = mybir.ActivationFunctionType.Gelu

    xt = nc.dram_tensor("xt", [D, C], dt.bfloat16, kind="ExternalInput")
    # w1 host-pretiled per-fo: w1t[fo, p, ko, j] = w1[ko*P+p, fo*P+j], so each
    # 256KB fo-tile is one contiguous-per-partition DMA and the PE can start
    # after the first tile instead of the full 8MB.
    w1t = nc.dram_tensor("w1t", [FO, P, KO, P], dt.bfloat16,
                         kind="ExternalInput")
    w2 = nc.dram_tensor("w2", [F, D], dt.bfloat16, kind="ExternalInput")
    # b1/sc pre-packed partition-major on host so each DMA is one contiguous
    # descriptor per partition (the rearranged 1-D loads were 4B-strided).
    b1p = nc.dram_tensor("b1p", [P, FO], dt.float32, kind="ExternalInput")
    b2r = nc.dram_tensor("b2r", [P, D], dt.float32, kind="ExternalInput")
    scp = nc.dram_tensor("scp", [P, C // P], dt.float32, kind="ExternalInput")
    y = nc.dram_tensor("y", [C, D], dt.float32, kind="ExternalOutput")

    blocks = []
    off = 0
    while off < C:
        size = min(NBLK, C - off)
        blocks.append((off, size))
        off += size

    xt_r = xt.rearrange("(ko p) c -> p ko c", p=P)

    with TileContext(nc) as tc:
        with (
            tc.tile_pool(name="w", bufs=1) as wpool,
            tc.tile_pool(name="xin", bufs=2) as xpool,
            tc.tile_pool(name="h", bufs=1) as hpool,
            tc.tile_pool(name="yout", bufs=2) as ypool,
            tc.tile_pool(name="ph", bufs=2, space="PSUM") as phpool,
            tc.tile_pool(name="py", bufs=4, space="PSUM") as pypool,
        ):
            # DMA issue order is the startup critical path: tiny consts,
            # then x for block 0, then w1 fo-tiles (in use order), then w2.
            b1sb = wpool.tile([P, FO], dt.float32, tag="b1")
            nc.sync.dma_start(b1sb[:], b1p[:])
            b2sb = wpool.tile([P, D], dt.float32, tag="b2")
            nc.sync.dma_start(b2sb[:], b2r[:])
            scsb = wpool.tile([P, C // P], dt.float32, tag="sc")
            nc.sync.dma_start(scsb[:], scp[:])

            def load_x_block(n_off, n_size):
                xts = []
                for ko in range(KO):
                    xt_t = xpool.tile([P, NBLK], dt.bfloat16, tag=f"x_{ko}")
                    nc.sync.dma_start(
                        xt_t[:, :n_size], xt_r[:, ko, n_off:n_off + n_size]
                    )
                    xts.append(xt_t)
                return xts

            xts0 = load_x_block(*blocks[0])

            w1sb = []
            for fo in range(FO):
                t_ = wpool.tile([P, KO, P], dt.bfloat16, tag=f"w1_{fo}")
                nc.sync.dma_start(t_[:], w1t[fo])
                w1sb.append(t_)
            w2sb = []
            for fo in range(FO):
                t_ = wpool.tile([P, D], dt.bfloat16, tag=f"w2_{fo}")
                nc.sync.dma_start(t_[:], w2[fo * P:(fo + 1) * P, :])
                w2sb.append(t_)

            for bi, (n_off, n_size) in enumerate(blocks):
                xts = xts0 if bi == 0 else load_x_block(n_off, n_size)

                # H^T[f, t] = sum_d w1[d, f] * x^T[d, t], then gelu(+b1).
                htile = hpool.tile([P, FO, NBLK], dt.bfloat16, tag="h")
                for fo in range(FO):
                    ph = phpool.tile([P, NBLK], dt.float32, tag="ph")
                    for ko in range(KO):
                        nc.tensor.matmul(
                            ph[:, :n_size],
                            w1sb[fo][:, ko, :],
                            xts[ko][:, :n_size],
                            start=(ko == 0),
                            stop=(ko == KO - 1),
                        )
                    nc.scalar.activation(
                        htile[:, fo, :n_size], ph[:, :n_size], GELU,
                        bias=b1sb[:, fo:fo + 1], scale=1.0,
                    )

                # Y[t, d] = sum_f H[t, f] * w2[f, d]; scale per token.
                for tb in range(n_size // P):
                    tbg = (n_off + tb * P) // P
                    ytile = ypool.tile([P, D], dt.float32, tag="y")
                    for dn in range(DN):
                        py = pypool.tile([P, NBLK], dt.float32, tag="py")
                        for fo in range(FO):
                            nc.tensor.matmul(
                                py[:],
                                htile[:, fo, tb * P:(tb + 1) * P],
                                w2sb[fo][:, dn * NBLK:(dn + 1) * NBLK],
                                start=(fo == 0),
                                stop=(fo == FO - 1),
                            )
                        nc.vector.tensor_add(
                            ytile[:, dn * NBLK:(dn + 1) * NBLK],
                            py[:],
                            b2sb[:, dn * NBLK:(dn + 1) * NBLK],
                        )
                    nc.vector.tensor_scalar_mul(
                        ytile[:], ytile[:], scsb[:, tbg:tbg + 1]
                    )
                    nc.sync.dma_start(
                        y[n_off + tb * P:n_off + (tb + 1) * P, :], ytile[:]
                    )
    nc.compile()
    return nc


def _route(flat, gate_w, gate_b):
    """fp32 gate matching the reference: softmax, top-2, renormalize."""
    logits = flat @ gate_w + gate_b
    m = logits.max(axis=1, keepdims=True)
    p = np.exp(logits - m, dtype=np.float32)
    probs = p / p.sum(axis=1, keepdims=True)
    ti = np.argsort(-probs, axis=1, kind="stable")[:, :TOPK]
    tp = np.take_along_axis(probs, ti, axis=1)
    sw = tp / (tp.sum(axis=1, keepdims=True) + np.float32(1e-9))
    return ti.astype(np.int64), sw.astype(np.float32)


def _dispatch(ti):
    """Slot assignment: (token, k) pair -> (expert, position-in-expert)."""
    Tn = ti.shape[0]
    flat_e = ti.ravel()
    order = np.argsort(flat_e, kind="stable")
    cnt = np.bincount(flat_e, minlength=E)
    starts = np.concatenate([[0], np.cumsum(cnt)[:-1]])
    ranks = np.arange(Tn * TOPK) - starts[flat_e[order]]
    pos = np.empty(Tn * TOPK, np.int64)
    pos[order] = ranks
    return flat_e, pos, cnt, starts, order


def kernel(**inputs) -> np.ndarray:
    global LAST
    x = np.asarray(inputs["x"], np.float32)
    gate_w = np.asarray(inputs["gate_w"], np.float32)
    gate_b = np.asarray(inputs["gate_b"], np.float32)
    w1 = np.asarray(inputs["w1"], np.float32)
    b1 = np.asarray(inputs["b1"], np.float32)
    w2 = np.asarray(inputs["w2"], np.float32)
    b2 = np.asarray(inputs["b2"], np.float32)

    B, S, D_ = x.shape
    flat = x.reshape(-1, D_)

    ti, sw = _route(flat, gate_w, gate_b)
    flat_e, pos, cnt, starts, order = _dispatch(ti)

    C = ((int(cnt.max()) + P - 1) // P) * P
    C = max(C, P)

    xT_bf = np.ascontiguousarray(flat.T).astype(_BF16)  # [D, T]
    sw_flat = sw.ravel()

    in_maps = []
    for e in range(E):
        pairs = order[starts[e]:starts[e] + cnt[e]]
        toks = pairs // TOPK
        xt_e = np.zeros((D, C), _BF16)
        xt_e[:, :cnt[e]] = xT_bf[:, toks]
        sc_e = np.zeros((C,), np.float32)
        sc_e[:cnt[e]] = sw_flat[pairs]
        KO, FO = D // P, F // P
        w1_tiled = np.ascontiguousarray(
            w1[e].astype(_BF16).reshape(KO, P, FO, P).transpose(2, 1, 0, 3)
        )
        in_maps.append({
            "xt": xt_e,
            "w1t": w1_tiled,
            "w2": w2[e].astype(_BF16),
            "b1p": np.ascontiguousarray(b1[e].reshape(F // P, P).T),
            "b2r": np.ascontiguousarray(
                np.broadcast_to(b2[e], (P, D))
            ).astype(np.float32),
            "scp": np.ascontiguousarray(sc_e.reshape(C // P, P).T),
        })

    nc = _nc_cache.get(C)
    if nc is None:
        nc = _build_moe_core(C)
        _nc_cache[C] = nc

    LAST = run_bass_kernel_spmd(nc, in_maps, core_ids=list(range(E)))
    Yall = np.stack([np.asarray(LAST.results[i]["y"]) for i in range(E)])

    contrib = Yall[flat_e, pos]  # [T*K, D] gather
    out = contrib[0::TOPK] + contrib[1::TOPK]
    return out.reshape(B, S, D_).astype(np.float32)
